# revision 7
# baseline (speedup 1.0000x reference)
"""Trainium2 Bass kernel: causal self-attention with RoPE.

Problem: x[4, 2048, 1024], W_qkv[3072, 1024], W_out[1024, 1024], 16 heads.
Sharding: 8 cores = (batch b, head-group hg of 8 heads); core c -> b=c//2,
hg=c%2. Each core computes a full [S, d_model] partial of the output (its
8 heads' contribution through out_proj); the host sums the two head-group
partials per batch.

On-chip layout is fully "transposed": q^T/k^T are produced as [d, s] tiles
(two heads per 128-partition tile), scores are computed as S^T = [k, q] so
the softmax needs no on-chip transposes, and PV/out_proj consume the
transposed forms directly, producing y in natural [s, e] layout.

V1 rework vs the original baseline:
- bf16 on-chip everywhere (qkT, v_aug, P, oc, weights); psum stays fp32.
- W_qkv/W_v/W_out resident in SBUF, loaded once at t=0 across 4 DMA queues.
- Warmup matmuls at t=0 keep the PE busy while the first DMAs land and
  prime the HAM clock gate (cold PE runs at 1.2 GHz, warm at 2.4 GHz).
- PV is split into two concurrent K=64 row-group matmuls (tile_position
  (0,0) / (64,0)) accumulating into separate psum tiles that are summed
  during evacuation; halves PV streaming time. Scores already ran the two
  heads of a pair concurrently the same way.
- Softmax normalization uses a DVE reciprocal on the 8 staged denominator
  rows + a K=8 selector matmul broadcast (no more ACT ln/exp round trip).
- Causal mask is a single [128,128] upper-triangle multiply on the diagonal
  128-block only (columns right of the diagonal block need no masking).
- One unified PSUM pool for the whole kernel: tag psA = 2x [128,1024]
  (scores / projection groups / out_proj / normalize broadcast / warmup),
  tag outT = 4x [65,512] PV accumulators. 8 banks exactly, no phase churn.
"""

import sys
import types
from contextlib import ExitStack

import numpy as np
import ml_dtypes

import concourse.bass as bass
import concourse.mybir as mybir
import concourse.tile as tile
from concourse import bacc, bass_utils

F32 = mybir.dt.float32
F32R = mybir.dt.float32r
BF16 = mybir.dt.bfloat16
AF = mybir.ActivationFunctionType

N_HEADS = 16
ROPE_BASE = 10000.0
B_FULL, S_FULL, DM = 4, 2048, 1024
HPC = 8          # heads per core
D = 64           # head dim
SCALE = 1.0 / 8.0  # D ** -0.5
SC = 512         # s-chunk width
KCN = DM // 128  # 8 contraction chunks for the projections
N_WARMUP = 18    # warmup matmuls (N=512 each) to cover initial DMA latency

PAIRSWAP = [i + 1 if i % 2 == 0 else i - 1 for i in range(32)]


def _install_ntff_hook_shim():
    """Register the axon NTFF profiling hook if antenv.axon_hooks is absent."""
    try:
        from antenv import axon_hooks  # noqa: F401
        return
    except ImportError:
        pass
    try:
        import antenv
        from trn_agent_boot.trn_boot import _ntff_profile_via_ctypes
        hook = _ntff_profile_via_ctypes('/opt/axon/libaxon_pjrt.so')
    except Exception:
        return
    mod = types.ModuleType('antenv.axon_hooks')
    mod._hook = hook
    mod.get_axon_ntff_profile_hook = lambda: mod._hook
    mod.set_axon_ntff_profile_hook = lambda h: setattr(mod, '_hook', h)
    sys.modules['antenv.axon_hooks'] = mod
    antenv.axon_hooks = mod


def build_program(s_len=S_FULL):
    """Build the single-core Bass program (identical across the 8 cores)."""
    nc = bacc.Bacc(None, target_bir_lowering=False, debug=False)

    xT = nc.dram_tensor("xT", [DM, s_len], BF16, kind="ExternalInput").ap()
    wqkT = nc.dram_tensor("wqkT", [DM, 1024], BF16, kind="ExternalInput").ap()
    wvT = nc.dram_tensor("wvT", [DM, 512], BF16, kind="ExternalInput").ap()
    woT = nc.dram_tensor("woT", [512, DM], BF16, kind="ExternalInput").ap()
    cosA = nc.dram_tensor("cosA", [128, s_len], F32, kind="ExternalInput").ap()
    sinA = nc.dram_tensor("sinA", [128, s_len], F32, kind="ExternalInput").ap()
    triH = nc.dram_tensor("triH", [128, 128], BF16, kind="ExternalInput").ap()
    ones8 = nc.dram_tensor("ones8", [128, 8], BF16, kind="ExternalInput").ap()
    pat8 = nc.dram_tensor("pat8", [8, 512], F32R, kind="ExternalInput").ap()
    y = nc.dram_tensor("y", [s_len, DM], F32, kind="ExternalOutput").ap()

    nsc = s_len // SC  # number of 512-wide s-chunks

    with tile.TileContext(nc) as tc:
        with ExitStack() as ctx, nc.allow_low_precision(reason="bf16 kernel"):
            pers = ctx.enter_context(tc.tile_pool(name="pers", bufs=1))
            ps_pool = ctx.enter_context(
                tc.tile_pool(name="ps", bufs=1, space="PSUM"))

            def psA(name):
                return ps_pool.tile([128, 1024], F32, tag="psA", bufs=2,
                                    name=name)

            qkT = [pers.tile([128, s_len], BF16, tag=f"qkT{t}", name=f"qkT{t}")
                   for t in range(8)]
            v_aug = [pers.tile([128, 8 * 65], BF16, tag=f"va{t}", name=f"va{t}")
                     for t in range(4 * nsc)]
            wqk = pers.tile([128, KCN * 1024], BF16, tag="wqk", name="wqk")
            wv = pers.tile([128, KCN * 512], BF16, tag="wv", name="wv")
            wo = pers.tile([128, 4 * 1024], BF16, tag="wo", name="wo")
            cosT = pers.tile([128, s_len], F32, tag="cos", name="cosT")
            sinT = pers.tile([128, s_len], F32, tag="sin", name="sinT")
            triT = pers.tile([128, 128], BF16, tag="tri", name="triT")
            pat8T = pers.tile([8, 512], F32R, tag="pat8", name="pat8T")
            wuS = pers.tile([128, 512], BF16, tag="wuS", name="wuS")

            # ---- warmup + initial DMAs (spread across engine queues) ----
            nc.vector.memset(wuS[:], 0.0)
            for i in range(N_WARMUP):
                wu = psA("wups")
                nc.tensor.matmul(wu[:, 0:512], wuS[:, 0:128], wuS[:],
                                 start=True, stop=True)

            for kc in range(KCN):
                nc.scalar.dma_start(wqk[:, 1024 * kc:1024 * (kc + 1)],
                                    wqkT[128 * kc:128 * (kc + 1), :])
            for k in range(4):
                nc.scalar.dma_start(wo[:, 1024 * k:1024 * (k + 1)],
                                    woT[128 * k:128 * (k + 1), :])
            for kc in range(KCN):
                nc.gpsimd.dma_start(wv[:, 512 * kc:512 * (kc + 1)],
                                    wvT[128 * kc:128 * (kc + 1), :])
            nc.scalar.dma_start(cosT[:], cosA[:])
            nc.scalar.dma_start(sinT[:], sinA[:])
            nc.gpsimd.dma_start(triT[:], triH[:])
            nc.gpsimd.dma_start(pat8T[:], pat8[:])
            for vt in range(4 * nsc):
                v3 = v_aug[vt][:].rearrange("p (h c) -> p h c", c=65)
                nc.gpsimd.dma_start(
                    v3[:, :, 64:65],
                    ones8[:].rearrange("p (h c) -> p h c", c=1))

            # ================= Phase 1: projections + RoPE =================
            with ExitStack() as pctx:
                xs_pool = pctx.enter_context(tc.tile_pool(name="xs", bufs=2))
                sh_pool = pctx.enter_context(tc.tile_pool(name="sh", bufs=3))

                xs = {}

                def load_xs(sc):
                    t = xs_pool.tile([128, KCN * 512], BF16, tag="xs",
                                     name="xs")
                    for kc in range(KCN):
                        nc.sync.dma_start(
                            t[:, 512 * kc:512 * (kc + 1)],
                            xT[128 * kc:128 * (kc + 1), SC * sc:SC * (sc + 1)])
                    xs[sc] = t

                load_xs(0)
                load_xs(1)

                for sc in range(nsc):
                    xt = xs[sc]
                    ssl = slice(SC * sc, SC * (sc + 1))
                    # q/k projections: 8 mg groups, 2 per psum tile
                    for gp in range(4):
                        ps = psA("pjqk")
                        for hc in range(2):
                            mg = 2 * gp + hc
                            half, mm = divmod(mg, 4)
                            c0 = 512 * hc
                            w0 = 512 * half + 128 * mm
                            for kc in range(KCN):
                                nc.tensor.matmul(
                                    ps[:, c0:c0 + 512],
                                    wqk[:, 1024 * kc + w0:1024 * kc + w0 + 128],
                                    xt[:, 512 * kc:512 * (kc + 1)],
                                    start=(kc == 0), stop=(kc == KCN - 1))
                        for hc in range(2):
                            mg = 2 * gp + hc
                            c0 = 512 * hc
                            # RoPE fold: qkT = ps*cos + pairswap(ps)*sin
                            shuf = sh_pool.tile([128, SC], F32, tag="sh",
                                                name="shuf")
                            nc.vector.stream_shuffle(shuf[:], ps[:, c0:c0 + 512],
                                                     PAIRSWAP)
                            nc.vector.tensor_mul(qkT[mg][:, ssl],
                                                 ps[:, c0:c0 + 512],
                                                 cosT[:, ssl])
                            nc.gpsimd.tensor_mul(shuf[:], shuf[:], sinT[:, ssl])
                            nc.vector.tensor_add(qkT[mg][:, ssl],
                                                 qkT[mg][:, ssl], shuf[:])
                    # v projection: 4 sv chunks, 2 per psum tile
                    for gv in range(2):
                        ps = psA("pjv")
                        for hc in range(2):
                            sv = 2 * gv + hc
                            c0 = 512 * hc
                            for kc in range(KCN):
                                nc.tensor.matmul(
                                    ps[:, c0:c0 + 512],
                                    xt[:, 512 * kc + 128 * sv:
                                       512 * kc + 128 * (sv + 1)],
                                    wv[:, 512 * kc:512 * (kc + 1)],
                                    start=(kc == 0), stop=(kc == KCN - 1))
                        for hc in range(2):
                            sv = 2 * gv + hc
                            vt = 4 * sc + sv
                            v3 = v_aug[vt][:].rearrange("p (h c) -> p h c",
                                                        c=65)
                            nc.scalar.copy(
                                v3[:, :, 0:64],
                                ps[:, 512 * hc:512 * (hc + 1)].rearrange(
                                    "p (h c) -> p h c", c=64))
                    if sc + 2 < nsc:
                        load_xs(sc + 2)

            # ================= Phase 2: attention + out_proj ===============
            with ExitStack() as actx:
                p_pool = actx.enter_context(tc.tile_pool(name="pp", bufs=5))
                ua_pool = actx.enter_context(tc.tile_pool(name="ua", bufs=2))
                oc_pool = actx.enter_context(tc.tile_pool(name="oc", bufs=8))
                ocu_pool = actx.enter_context(tc.tile_pool(name="ocu", bufs=2))
                rc_pool = actx.enter_context(tc.tile_pool(name="rc", bufs=2))
                y_pool = actx.enter_context(tc.tile_pool(name="yst", bufs=3))

                def attention_qc(qc, ocU_all, fillers):
                    """All 4 head pairs of q-chunk qc, flattened (pair, kb)
                    stream with scores emitted 1 step ahead, deferred
                    out_proj chunks dripped in as PE filler. PV runs as two
                    concurrent K=64 row-group matmuls per head."""
                    nblk = 4 * qc + 4
                    outT = {}
                    sc_ps = {}

                    def q0_of(kb):
                        j = kb - 4 * qc
                        return 128 * j if j >= 0 else 0

                    def emit_scores(p, kb):
                        qT, kT = qkT[p], qkT[4 + p]
                        q0 = q0_of(kb)
                        ksl = slice(128 * kb, 128 * (kb + 1))
                        ps = psA("scps")
                        nc.tensor.matmul(
                            ps[:, q0:512], kT[0:64, ksl],
                            qT[0:64, SC * qc + q0:SC * (qc + 1)],
                            start=True, stop=True, tile_position=(0, 0))
                        nc.tensor.matmul(
                            ps[:, 512 + q0:1024], kT[64:128, ksl],
                            qT[64:128, SC * qc + q0:SC * (qc + 1)],
                            start=True, stop=True, tile_position=(64, 0))
                        sc_ps[p, kb] = ps

                    def emit_softmax_pv(p, kb):
                        q0 = q0_of(kb)
                        j = kb - 4 * qc
                        ps = sc_ps.pop((p, kb))
                        if kb == 0:
                            for h in range(2):
                                for hf in range(2):
                                    outT[p, h, hf] = ps_pool.tile(
                                        [65, SC], F32, tag="outT", bufs=4,
                                        name="outps")
                        P = p_pool.tile([128, 1024], BF16, tag="P", name="Pt")
                        vps = ps[:].rearrange("p (two q) -> p two q", two=2)
                        vP = P[:].rearrange("p (two q) -> p two q", two=2)
                        nc.scalar.activation(vP[:, :, q0:512], vps[:, :, q0:512],
                                             AF.Exp, scale=SCALE)
                        if j >= 0:
                            # mask only the diagonal 128x128 block
                            nc.vector.tensor_mul(P[:, q0:q0 + 128],
                                                 P[:, q0:q0 + 128], triT[:])
                            nc.vector.tensor_mul(P[:, 512 + q0:512 + q0 + 128],
                                                 P[:, 512 + q0:512 + q0 + 128],
                                                 triT[:])
                        va = v_aug[kb]
                        for h in range(2):
                            pc0 = 512 * h
                            vc0 = 130 * p + 65 * h
                            nc.tensor.matmul(
                                outT[p, h, 0][:, q0:512],
                                va[0:64, vc0:vc0 + 65],
                                P[0:64, pc0 + q0:pc0 + 512],
                                start=(kb == 0), stop=(kb == nblk - 1),
                                tile_position=(0, 0))
                            nc.tensor.matmul(
                                outT[p, h, 1][:, q0:512],
                                va[64:128, vc0:vc0 + 65],
                                P[64:128, pc0 + q0:pc0 + 512],
                                start=(kb == 0), stop=(kb == nblk - 1),
                                tile_position=(64, 0))
                        if kb == nblk - 1:
                            for h in range(2):
                                i = 2 * p + h
                                # DVE reads at most one PSUM operand: stage
                                # half A in SBUF fp32, then add half B.
                                ua = ua_pool.tile([65, SC], F32, tag="ua",
                                                  name="ua")
                                nc.vector.tensor_copy(
                                    ua[:], outT.pop((p, h, 0))[:])
                                nc.vector.tensor_add(
                                    ocU_all[:, 512 * i:512 * (i + 1)],
                                    ua[:], outT.pop((p, h, 1))[:])

                    stream = [(p, kb) for p in range(4) for kb in range(nblk)]
                    emitted = 0
                    for idx, (p, kb) in enumerate(stream):
                        while emitted <= idx + 1 and emitted < len(stream):
                            emit_scores(*stream[emitted])
                            emitted += 1
                        emit_softmax_pv(p, kb)
                        if fillers and idx % 3 == 2:
                            fillers.pop(0)()

                def normalize(qc, ocU_all):
                    """DVE reciprocal on the 8 staged denominator rows, then
                    K=8 selector matmul broadcast + the normalize muls."""
                    dn8 = rc_pool.tile([8, SC], BF16, tag="dn8", name="dn8")
                    for i in range(8):
                        nc.gpsimd.dma_start(dn8[i:i + 1, :],
                                            ocU_all[64:65,
                                                    512 * i:512 * (i + 1)])
                    rcp = rc_pool.tile([8, SC], F32R, tag="rcp", name="rcp")
                    nc.vector.reciprocal(rcp[:], dn8[:])
                    oc_t = [oc_pool.tile([128, SC], BF16, tag="oc", name="oc")
                            for _ in range(4)]
                    for p in range(4):
                        bcq = psA("bcq")
                        nc.tensor.matmul(bcq[:, 0:SC],
                                         pat8T[:, 128 * p:128 * (p + 1)],
                                         rcp[:], start=True, stop=True)
                        nc.vector.tensor_mul(
                            oc_t[p][0:64, :],
                            ocU_all[0:64, 1024 * p:1024 * p + 512],
                            bcq[0:64, 0:SC])
                        nc.vector.tensor_mul(
                            oc_t[p][64:128, :],
                            ocU_all[0:64, 1024 * p + 512:1024 * (p + 1)],
                            bcq[64:128, 0:SC])
                    return oc_t

                def outproj_chunk(qc, oc_t, sv):
                    svsl = slice(128 * sv, 128 * (sv + 1))
                    ps = psA("psy")
                    for k in range(4):
                        nc.tensor.matmul(ps[:, 0:512], oc_t[k][:, svsl],
                                         wo[:, 1024 * k:1024 * k + 512],
                                         start=(k == 0), stop=(k == 3))
                        nc.tensor.matmul(ps[:, 512:1024], oc_t[k][:, svsl],
                                         wo[:, 1024 * k + 512:1024 * (k + 1)],
                                         start=(k == 0), stop=(k == 3))
                    yt = y_pool.tile([128, 1024], F32, tag="yst", name="yt")
                    nc.vector.tensor_copy(yt[:], ps[:])
                    nc.sync.dma_start(
                        y[SC * qc + 128 * sv:SC * qc + 128 * (sv + 1), :],
                        yt[:])

                pending = None
                fillers = []
                for qc in range(nsc):
                    ocU_all = ocu_pool.tile([65, 8 * SC], BF16, tag="ocu",
                                            name="ocu")
                    if pending is not None:
                        pqc, pocU = pending
                        oc_t = normalize(pqc, pocU)
                        fillers.extend(
                            (lambda sv=sv, q=pqc, o=oc_t:
                             outproj_chunk(q, o, sv)) for sv in range(4))
                        pending = None
                    attention_qc(qc, ocU_all, fillers)
                    pending = (qc, ocU_all)
                for fn in fillers:
                    fn()
                pqc, pocU = pending
                oc_t = normalize(pqc, pocU)
                for sv in range(4):
                    outproj_chunk(pqc, oc_t, sv)

    nc.compile()
    return nc


# ---------------------------------------------------------------------------
# Host-side input preparation
# ---------------------------------------------------------------------------

BF = ml_dtypes.bfloat16


def _rope_tables(s_len):
    perm = np.empty(64, dtype=np.int64)
    perm[0::2] = np.arange(32)
    perm[1::2] = np.arange(32) + 32
    inv_freq = 1.0 / (ROPE_BASE ** (np.arange(0, D, 2, dtype=np.float32) / D))
    t = np.arange(s_len, dtype=np.float32)
    freqs = np.einsum('i,j->ij', t, inv_freq)           # [S, 32]
    emb = np.concatenate([freqs, freqs], axis=-1)       # [S, 64]
    cos = np.cos(emb).T.astype(np.float32)              # [64, S]
    sin = np.sin(emb).T.astype(np.float32)
    cos64 = cos[perm]
    sin64 = sin[perm]
    sign = np.where(perm < 32, -1.0, 1.0).astype(np.float32)[:, None]
    sin64 = sin64 * sign
    cosA = np.ascontiguousarray(np.tile(cos64, (2, 1)))
    sinA = np.ascontiguousarray(np.tile(sin64, (2, 1)))
    return perm, cosA, sinA


def make_in_maps(x, W_qkv, W_out, s_len=S_FULL):
    B = x.shape[0]
    perm, cosA, sinA = _rope_tables(s_len)
    tri = np.triu(np.ones((128, 128), dtype=np.float32)).astype(BF)
    pat = np.zeros((8, 512), dtype=np.float32)
    for p in range(4):
        pat[2 * p, 128 * p:128 * p + 64] = 1.0
        pat[2 * p + 1, 128 * p + 64:128 * (p + 1)] = 1.0
    in_maps = []
    for c in range(2 * B):
        b, hg = c // 2, c % 2
        xTb = np.ascontiguousarray(x[b, :s_len].T.astype(BF))
        cols = []
        for h in range(HPC):
            cols.append(W_qkv[64 * (HPC * hg + h) + perm])          # q head
        for h in range(HPC):
            cols.append(W_qkv[1024 + 64 * (HPC * hg + h) + perm])   # k head
        wqkT = np.ascontiguousarray(
            np.concatenate(cols, axis=0).T.astype(BF))
        wvT = np.ascontiguousarray(
            W_qkv[2048 + 512 * hg:2048 + 512 * (hg + 1)].T.astype(BF))
        woT = np.ascontiguousarray(
            W_out[:, 512 * hg:512 * (hg + 1)].T.astype(BF))
        in_maps.append({
            "xT": xTb, "wqkT": wqkT, "wvT": wvT, "woT": woT,
            "cosA": cosA, "sinA": sinA, "triH": tri,
            "ones8": np.ones((128, 8), dtype=BF),
            "pat8": pat,
        })
    return in_maps


_NC_CACHE = {}


def _get_program(s_len=S_FULL):
    if s_len not in _NC_CACHE:
        _NC_CACHE[s_len] = build_program(s_len)
    return _NC_CACHE[s_len]


def kernel(x, W_qkv, W_out):
    """Full-input, full-output causal self-attention on 8 NeuronCores."""
    _install_ntff_hook_shim()
    x = np.asarray(x, dtype=np.float32)
    W_qkv = np.asarray(W_qkv, dtype=np.float32)
    W_out = np.asarray(W_out, dtype=np.float32)
    B, S, dm = x.shape

    nc = _get_program(S)
    in_maps = make_in_maps(x, W_qkv, W_out, S)
    res = bass_utils.run_bass_kernel_spmd(nc, in_maps, list(range(2 * B)))
    out = np.empty((B, S, dm), dtype=np.float32)
    for b in range(B):
        out[b] = res.results[2 * b]["y"] + res.results[2 * b + 1]["y"]
    return out


# revision 8
# speedup vs baseline: 1.2177x; 1.2177x over previous
"""Trainium2 Bass kernel: causal self-attention with RoPE.

Problem: x[4, 2048, 1024], W_qkv[3072, 1024], W_out[1024, 1024], 16 heads.
Sharding: 8 cores = (batch b, head-group hg of 8 heads); core c -> b=c//2,
hg=c%2. Each core computes a full [S, d_model] partial of the output (its
8 heads' contribution through out_proj); the host sums the two head-group
partials per batch.

On-chip layout is fully "transposed": q^T/k^T are produced as [d, s] tiles
(two heads per 128-partition tile), scores are computed as S^T = [k, q] so
the softmax needs no on-chip transposes, and PV/out_proj consume the
transposed forms directly, producing y in natural [s, e] layout.

V2: single fused pipeline. The QKV projection for s-chunk sc+1, the
normalize/out_proj of chunk qc-1, and the x-prefetch DMAs are dripped as
"filler" work units between the attention steps of chunk qc, so the PE
never sits idle while ACT runs the softmax exps (which are the true
bottleneck: ACT is 1 elem/cycle/lane @ 1.2 GHz, ~160us of exp total).
Everything on-chip is bf16 except psum/rope/y staging; weights live
resident in SBUF and all big inputs are host-prelaid so each load is one
fat-row DMA descriptor. Warmup matmuls at t=0 prime the HAM clock gate.
"""

import sys
import types
from collections import deque
from contextlib import ExitStack

import numpy as np
import ml_dtypes

import concourse.bass as bass
import concourse.mybir as mybir
import concourse.tile as tile
from concourse import bacc, bass_utils

F32 = mybir.dt.float32
F32R = mybir.dt.float32r
BF16 = mybir.dt.bfloat16
AF = mybir.ActivationFunctionType

N_HEADS = 16
ROPE_BASE = 10000.0
B_FULL, S_FULL, DM = 4, 2048, 1024
HPC = 8          # heads per core
D = 64           # head dim
SCALE = 1.0 / 8.0  # D ** -0.5
SC = 512         # s-chunk width
KCN = DM // 128  # 8 contraction chunks for the projections
N_WARMUP = 10

PAIRSWAP = [i + 1 if i % 2 == 0 else i - 1 for i in range(32)]


def _install_ntff_hook_shim():
    """Register the axon NTFF profiling hook if antenv.axon_hooks is absent."""
    try:
        from antenv import axon_hooks  # noqa: F401
        return
    except ImportError:
        pass
    try:
        import antenv
        from trn_agent_boot.trn_boot import _ntff_profile_via_ctypes
        hook = _ntff_profile_via_ctypes('/opt/axon/libaxon_pjrt.so')
    except Exception:
        return
    mod = types.ModuleType('antenv.axon_hooks')
    mod._hook = hook
    mod.get_axon_ntff_profile_hook = lambda: mod._hook
    mod.set_axon_ntff_profile_hook = lambda h: setattr(mod, '_hook', h)
    sys.modules['antenv.axon_hooks'] = mod
    antenv.axon_hooks = mod


def build_program(s_len=S_FULL):
    """Build the single-core Bass program (identical across the 8 cores)."""
    nc = bacc.Bacc(None, target_bir_lowering=False, debug=False)

    nsc = s_len // SC  # number of 512-wide s-chunks

    # Host-prelaid: fat contiguous rows so each load is ONE DMA descriptor.
    xsH = nc.dram_tensor("xsH", [128, nsc * KCN * 512], BF16,
                         kind="ExternalInput").ap()
    wqkH = nc.dram_tensor("wqkH", [128, KCN * 1024], BF16,
                          kind="ExternalInput").ap()
    wvH = nc.dram_tensor("wvH", [128, KCN * 512], BF16,
                         kind="ExternalInput").ap()
    woH = nc.dram_tensor("woH", [128, 4 * 1024], BF16,
                         kind="ExternalInput").ap()
    cosA = nc.dram_tensor("cosA", [128, s_len], F32, kind="ExternalInput").ap()
    sinA = nc.dram_tensor("sinA", [128, s_len], F32, kind="ExternalInput").ap()
    triH = nc.dram_tensor("triH", [128, 128], BF16, kind="ExternalInput").ap()
    pat8 = nc.dram_tensor("pat8", [8, 512], F32R, kind="ExternalInput").ap()
    y = nc.dram_tensor("y", [s_len, DM], F32, kind="ExternalOutput").ap()

    with tile.TileContext(nc) as tc:
        with ExitStack() as ctx, nc.allow_low_precision(reason="bf16 kernel"):
            pers = ctx.enter_context(tc.tile_pool(name="pers", bufs=1))
            ps_pool = ctx.enter_context(
                tc.tile_pool(name="ps", bufs=1, space="PSUM"))
            xs_pool = ctx.enter_context(tc.tile_pool(name="xs", bufs=2))
            sh_pool = ctx.enter_context(tc.tile_pool(name="sh", bufs=3))
            p_pool = ctx.enter_context(tc.tile_pool(name="pp", bufs=5))
            oc_pool = ctx.enter_context(tc.tile_pool(name="oc", bufs=8))
            ocu_pool = ctx.enter_context(tc.tile_pool(name="ocu", bufs=2))
            rc_pool = ctx.enter_context(tc.tile_pool(name="rc", bufs=2))
            y_pool = ctx.enter_context(tc.tile_pool(name="yst", bufs=3))

            def psA(name):
                return ps_pool.tile([128, 1024], F32, tag="psA", bufs=3,
                                    name=name)

            qkT = [pers.tile([128, s_len], BF16, tag=f"qkT{t}", name=f"qkT{t}")
                   for t in range(8)]
            v_aug = [pers.tile([128, 8 * 65], BF16, tag=f"va{t}", name=f"va{t}")
                     for t in range(4 * nsc)]
            wqk = pers.tile([128, KCN * 1024], BF16, tag="wqk", name="wqk")
            wv = pers.tile([128, KCN * 512], BF16, tag="wv", name="wv")
            wo = pers.tile([128, 4 * 1024], BF16, tag="wo", name="wo")
            cosT = pers.tile([128, s_len], F32, tag="cos", name="cosT")
            sinT = pers.tile([128, s_len], F32, tag="sin", name="sinT")
            triT = pers.tile([128, 128], BF16, tag="tri", name="triT")
            pat8T = pers.tile([8, 512], F32R, tag="pat8", name="pat8T")
            wuS = pers.tile([128, 512], BF16, tag="wuS", name="wuS")
            junk = pers.tile([1, 16], BF16, tag="junk", name="junk")

            # ---- warmup (prime HAM + cover initial DMA latency) ----
            nc.vector.memset(wuS[:], 0.0)
            for i in range(N_WARMUP):
                wu = psA("wups")
                nc.tensor.matmul(wu[:, 0:512], wuS[:, 0:128], wuS[:],
                                 start=True, stop=True)
            # preload the ACT exp table during proj(0)
            nc.scalar.activation(junk[:], wuS[0:1, 0:16], AF.Exp, scale=SCALE)

            # ---- initial DMAs, priority-ordered per queue ----
            xs = {}

            def load_xs(sc):
                t = xs_pool.tile([128, KCN * 512], BF16, tag="xs", name="xs")
                nc.sync.dma_start(
                    t[:], xsH[:, 4096 * sc:4096 * (sc + 1)])
                xs[sc] = t

            load_xs(0)
            load_xs(1)
            nc.scalar.dma_start(wqk[:], wqkH[:])
            nc.scalar.dma_start(cosT[:], cosA[:])
            nc.scalar.dma_start(sinT[:], sinA[:])
            nc.gpsimd.dma_start(triT[:], triH[:])
            nc.gpsimd.dma_start(pat8T[:], pat8[:])
            nc.gpsimd.dma_start(wv[:], wvH[:])
            nc.gpsimd.dma_start(wo[:], woH[:])
            for vt in range(4 * nsc):
                v3 = v_aug[vt][:].rearrange("p (h c) -> p h c", c=65)
                nc.vector.memset(v3[:, :, 64:65], 1.0)

            # ---- projection work units ----
            def qk_unit(sc, gp):
                """One psum tile: projections for head-pair groups
                mg=2gp, 2gp+1 of s-chunk sc, plus their RoPE evacuation."""
                xt = xs[sc]
                ssl = slice(SC * sc, SC * (sc + 1))
                ps = psA("pjqk")
                for hc in range(2):
                    mg = 2 * gp + hc
                    half, mm = divmod(mg, 4)
                    c0 = 512 * hc
                    w0 = 512 * half + 128 * mm
                    for kc in range(KCN):
                        nc.tensor.matmul(
                            ps[:, c0:c0 + 512],
                            wqk[:, 1024 * kc + w0:1024 * kc + w0 + 128],
                            xt[:, 512 * kc:512 * (kc + 1)],
                            start=(kc == 0), stop=(kc == KCN - 1))
                for hc in range(2):
                    mg = 2 * gp + hc
                    c0 = 512 * hc
                    shuf = sh_pool.tile([128, SC], F32, tag="sh", name="shuf")
                    nc.vector.stream_shuffle(shuf[:], ps[:, c0:c0 + 512],
                                             PAIRSWAP)
                    nc.vector.tensor_mul(qkT[mg][:, ssl], ps[:, c0:c0 + 512],
                                         cosT[:, ssl])
                    nc.gpsimd.tensor_mul(shuf[:], shuf[:], sinT[:, ssl])
                    nc.vector.tensor_add(qkT[mg][:, ssl], qkT[mg][:, ssl],
                                         shuf[:])

            def v_unit(sc, gv):
                """One psum tile: v projection for sv=2gv, 2gv+1 of chunk sc."""
                xt = xs[sc]
                ps = psA("pjv")
                for hc in range(2):
                    sv = 2 * gv + hc
                    c0 = 512 * hc
                    for kc in range(KCN):
                        nc.tensor.matmul(
                            ps[:, c0:c0 + 512],
                            xt[:, 512 * kc + 128 * sv:
                               512 * kc + 128 * (sv + 1)],
                            wv[:, 512 * kc:512 * (kc + 1)],
                            start=(kc == 0), stop=(kc == KCN - 1))
                for hc in range(2):
                    sv = 2 * gv + hc
                    vt = 4 * sc + sv
                    v3 = v_aug[vt][:].rearrange("p (h c) -> p h c", c=65)
                    nc.scalar.copy(
                        v3[:, :, 0:64],
                        ps[:, 512 * hc:512 * (hc + 1)].rearrange(
                            "p (h c) -> p h c", c=64))

            def proj_units(sc):
                us = [lambda gp=gp: qk_unit(sc, gp) for gp in range(4)]
                us += [lambda gv=gv: v_unit(sc, gv) for gv in range(2)]
                return us

            # ---- attention ----
            def attention_qc(qc, ocU_all, fillers):
                nblk = 4 * qc + 4
                outT = {}
                sc_ps = {}

                def q0_of(kb):
                    j = kb - 4 * qc
                    return 128 * j if j >= 0 else 0

                def emit_scores(p, kb):
                    qT, kT = qkT[p], qkT[4 + p]
                    q0 = q0_of(kb)
                    ksl = slice(128 * kb, 128 * (kb + 1))
                    ps = psA("scps")
                    nc.tensor.matmul(
                        ps[:, q0:512], kT[0:64, ksl],
                        qT[0:64, SC * qc + q0:SC * (qc + 1)],
                        start=True, stop=True, tile_position=(0, 0))
                    nc.tensor.matmul(
                        ps[:, 512 + q0:1024], kT[64:128, ksl],
                        qT[64:128, SC * qc + q0:SC * (qc + 1)],
                        start=True, stop=True, tile_position=(64, 0))
                    sc_ps[p, kb] = ps

                def emit_softmax_pv(p, kb):
                    q0 = q0_of(kb)
                    j = kb - 4 * qc
                    ps = sc_ps.pop((p, kb))
                    if kb == 0:
                        outT[p, 0] = ps_pool.tile([65, SC], F32, tag="outT",
                                                  bufs=2, name="outA")
                        outT[p, 1] = ps_pool.tile([65, SC], F32, tag="outT",
                                                  bufs=2, name="outB")
                    P = p_pool.tile([128, 1024], BF16, tag="P", name="Pt")
                    vps = ps[:].rearrange("p (two q) -> p two q", two=2)
                    vP = P[:].rearrange("p (two q) -> p two q", two=2)
                    nc.scalar.activation(vP[:, :, q0:512], vps[:, :, q0:512],
                                         AF.Exp, scale=SCALE)
                    if j >= 0:
                        # mask only the diagonal 128x128 block
                        nc.vector.tensor_mul(P[:, q0:q0 + 128],
                                             P[:, q0:q0 + 128], triT[:])
                        nc.vector.tensor_mul(P[:, 512 + q0:512 + q0 + 128],
                                             P[:, 512 + q0:512 + q0 + 128],
                                             triT[:])
                    va = v_aug[kb]
                    nc.tensor.matmul(
                        outT[p, 0][:, q0:512], va[:, 130 * p:130 * p + 65],
                        P[:, q0:512],
                        start=(kb == 0), stop=(kb == nblk - 1))
                    nc.tensor.matmul(
                        outT[p, 1][:, q0:512],
                        va[:, 130 * p + 65:130 * p + 130],
                        P[:, 512 + q0:1024],
                        start=(kb == 0), stop=(kb == nblk - 1))
                    if kb == nblk - 1:
                        for h in range(2):
                            i = 2 * p + h
                            nc.vector.tensor_copy(
                                ocU_all[:, 512 * i:512 * (i + 1)],
                                outT.pop((p, h))[:])

                stream = [(p, kb) for p in range(4) for kb in range(nblk)]
                iv = max(2, (len(stream) + len(fillers)) // (len(fillers) + 1)) \
                    if fillers else 10 ** 9
                emitted = 0
                for idx, (p, kb) in enumerate(stream):
                    while emitted <= idx + 2 and emitted < len(stream):
                        emit_scores(*stream[emitted])
                        emitted += 1
                    emit_softmax_pv(p, kb)
                    if fillers and idx % iv == iv - 1:
                        fillers.popleft()()

            def normalize(qc, ocU_all):
                """DVE reciprocal of the 8 staged denominator rows, then a
                K=8 selector matmul broadcast + the normalize muls."""
                dn8 = rc_pool.tile([8, SC], BF16, tag="dn8", name="dn8")
                for i in range(8):
                    nc.gpsimd.dma_start(dn8[i:i + 1, :],
                                        ocU_all[64:65, 512 * i:512 * (i + 1)])
                dnf = rc_pool.tile([8, SC], F32, tag="dnf", name="dnf")
                nc.vector.tensor_copy(dnf[:], dn8[:])
                rcp = rc_pool.tile([8, SC], F32, tag="rcp", name="rcp")
                nc.vector.reciprocal_approx_fast(rcp[:], dnf[:])
                rcpR = rc_pool.tile([8, SC], F32R, tag="rcpR", name="rcpR")
                nc.vector.tensor_copy(rcpR[:], rcp[:])
                oc_t = [oc_pool.tile([128, SC], BF16, tag="oc", name="oc")
                        for _ in range(4)]
                for p in range(4):
                    bcq = psA("bcq")
                    nc.tensor.matmul(bcq[:, 0:SC],
                                     pat8T[:, 128 * p:128 * (p + 1)],
                                     rcpR[:], start=True, stop=True)
                    nc.vector.tensor_mul(
                        oc_t[p][0:64, :],
                        ocU_all[0:64, 1024 * p:1024 * p + 512],
                        bcq[0:64, 0:SC])
                    nc.vector.tensor_mul(
                        oc_t[p][64:128, :],
                        ocU_all[0:64, 1024 * p + 512:1024 * (p + 1)],
                        bcq[64:128, 0:SC])
                return oc_t

            def outproj_chunk(qc, oc_t, sv):
                svsl = slice(128 * sv, 128 * (sv + 1))
                ps = psA("psy")
                for k in range(4):
                    nc.tensor.matmul(ps[:, 0:512], oc_t[k][:, svsl],
                                     wo[:, 1024 * k:1024 * k + 512],
                                     start=(k == 0), stop=(k == 3))
                    nc.tensor.matmul(ps[:, 512:1024], oc_t[k][:, svsl],
                                     wo[:, 1024 * k + 512:1024 * (k + 1)],
                                     start=(k == 0), stop=(k == 3))
                yt = y_pool.tile([128, 1024], F32, tag="yst", name="yt")
                nc.vector.tensor_copy(yt[:], ps[:])
                nc.sync.dma_start(
                    y[SC * qc + 128 * sv:SC * qc + 128 * (sv + 1), :],
                    yt[:])

            # ---- fused schedule ----
            for u in proj_units(0):
                u()
            fillers = deque()
            pending = None
            for qc in range(nsc):
                if qc + 2 < nsc:
                    fillers.append(lambda sc=qc + 2: load_xs(sc))
                if pending is not None:
                    pqc, pocU = pending
                    oc_t = normalize(pqc, pocU)
                    fillers.extend(
                        (lambda sv=sv, q=pqc, o=oc_t:
                         outproj_chunk(q, o, sv)) for sv in range(4))
                    pending = None
                if qc + 1 < nsc:
                    fillers.extend(proj_units(qc + 1))
                ocU_all = ocu_pool.tile([65, 8 * SC], BF16, tag="ocu",
                                        name="ocu")
                attention_qc(qc, ocU_all, fillers)
                pending = (qc, ocU_all)
            while fillers:
                fillers.popleft()()
            pqc, pocU = pending
            oc_t = normalize(pqc, pocU)
            for sv in range(4):
                outproj_chunk(pqc, oc_t, sv)

    nc.compile()
    return nc


# ---------------------------------------------------------------------------
# Host-side input preparation
# ---------------------------------------------------------------------------

BF = ml_dtypes.bfloat16


def _rope_tables(s_len):
    perm = np.empty(64, dtype=np.int64)
    perm[0::2] = np.arange(32)
    perm[1::2] = np.arange(32) + 32
    inv_freq = 1.0 / (ROPE_BASE ** (np.arange(0, D, 2, dtype=np.float32) / D))
    t = np.arange(s_len, dtype=np.float32)
    freqs = np.einsum('i,j->ij', t, inv_freq)           # [S, 32]
    emb = np.concatenate([freqs, freqs], axis=-1)       # [S, 64]
    cos = np.cos(emb).T.astype(np.float32)              # [64, S]
    sin = np.sin(emb).T.astype(np.float32)
    cos64 = cos[perm]
    sin64 = sin[perm]
    sign = np.where(perm < 32, -1.0, 1.0).astype(np.float32)[:, None]
    sin64 = sin64 * sign
    cosA = np.ascontiguousarray(np.tile(cos64, (2, 1)))
    sinA = np.ascontiguousarray(np.tile(sin64, (2, 1)))
    return perm, cosA, sinA


def _chunk128(a):
    """[N*128, M] -> [128, N*M] with N-major column blocks."""
    n = a.shape[0] // 128
    return np.ascontiguousarray(
        a.reshape(n, 128, a.shape[1]).transpose(1, 0, 2).reshape(
            128, n * a.shape[1]))


def make_in_maps(x, W_qkv, W_out, s_len=S_FULL):
    B = x.shape[0]
    nsc = s_len // SC
    perm, cosA, sinA = _rope_tables(s_len)
    tri = np.triu(np.ones((128, 128), dtype=np.float32)).astype(BF)
    pat = np.zeros((8, 512), dtype=np.float32)
    for p in range(4):
        pat[2 * p, 128 * p:128 * p + 64] = 1.0
        pat[2 * p + 1, 128 * p + 64:128 * (p + 1)] = 1.0
    in_maps = []
    for c in range(2 * B):
        b, hg = c // 2, c % 2
        # xsH[p, sc*4096 + kc*512 + q] = x[b, 512*sc + q, 128*kc + p]
        xb = x[b, :s_len].reshape(nsc, 512, KCN, 128)     # [sc, q, kc, p]
        xsH = np.ascontiguousarray(
            xb.transpose(3, 0, 2, 1).reshape(128, nsc * KCN * 512).astype(BF))
        cols = []
        for h in range(HPC):
            cols.append(W_qkv[64 * (HPC * hg + h) + perm])          # q head
        for h in range(HPC):
            cols.append(W_qkv[1024 + 64 * (HPC * hg + h) + perm])   # k head
        wqkT = np.concatenate(cols, axis=0).T.astype(BF)  # [1024, 1024]
        wvT = W_qkv[2048 + 512 * hg:2048 + 512 * (hg + 1)].T.astype(BF)
        woT = W_out[:, 512 * hg:512 * (hg + 1)].T.astype(BF)
        in_maps.append({
            "xsH": xsH,
            "wqkH": _chunk128(wqkT),
            "wvH": _chunk128(wvT),
            "woH": _chunk128(woT),
            "cosA": cosA, "sinA": sinA, "triH": tri,
            "pat8": pat,
        })
    return in_maps


_NC_CACHE = {}


def _get_program(s_len=S_FULL):
    if s_len not in _NC_CACHE:
        _NC_CACHE[s_len] = build_program(s_len)
    return _NC_CACHE[s_len]


def kernel(x, W_qkv, W_out):
    """Full-input, full-output causal self-attention on 8 NeuronCores."""
    _install_ntff_hook_shim()
    x = np.asarray(x, dtype=np.float32)
    W_qkv = np.asarray(W_qkv, dtype=np.float32)
    W_out = np.asarray(W_out, dtype=np.float32)
    B, S, dm = x.shape

    nc = _get_program(S)
    in_maps = make_in_maps(x, W_qkv, W_out, S)
    res = bass_utils.run_bass_kernel_spmd(nc, in_maps, list(range(2 * B)))
    out = np.empty((B, S, dm), dtype=np.float32)
    for b in range(B):
        out[b] = res.results[2 * b]["y"] + res.results[2 * b + 1]["y"]
    return out


# revision 9
# speedup vs baseline: 1.2409x; 1.0191x over previous
"""Trainium2 Bass kernel: causal self-attention with RoPE.

Problem: x[4, 2048, 1024], W_qkv[3072, 1024], W_out[1024, 1024], 16 heads.
Sharding: 8 cores = (batch b, head-group hg of 8 heads); core c -> b=c//2,
hg=c%2. Each core computes a full [S, d_model] partial of the output (its
8 heads' contribution through out_proj); the host sums the two head-group
partials per batch.

On-chip layout is fully "transposed": q^T/k^T are produced as [d, s] tiles
(two heads per 128-partition tile), scores are computed as S^T = [k, q] so
the softmax needs no on-chip transposes, and PV/out_proj consume the
transposed forms directly, producing y in natural [s, e] layout.

V2: single fused pipeline. The QKV projection for s-chunk sc+1, the
normalize/out_proj of chunk qc-1, and the x-prefetch DMAs are dripped as
"filler" work units between the attention steps of chunk qc, so the PE
never sits idle while ACT runs the softmax exps (which are the true
bottleneck: ACT is 1 elem/cycle/lane @ 1.2 GHz, ~160us of exp total).
Everything on-chip is bf16 except psum/rope/y staging; weights live
resident in SBUF and all big inputs are host-prelaid so each load is one
fat-row DMA descriptor. Warmup matmuls at t=0 prime the HAM clock gate.
"""

import sys
import types
from collections import deque
from contextlib import ExitStack

import numpy as np
import ml_dtypes

import concourse.bass as bass
import concourse.mybir as mybir
import concourse.tile as tile
from concourse import bacc, bass_utils

F32 = mybir.dt.float32
F32R = mybir.dt.float32r
BF16 = mybir.dt.bfloat16
AF = mybir.ActivationFunctionType

N_HEADS = 16
ROPE_BASE = 10000.0
B_FULL, S_FULL, DM = 4, 2048, 1024
HPC = 8          # heads per core
D = 64           # head dim
SCALE = 1.0 / 8.0  # D ** -0.5
SC = 512         # s-chunk width
KCN = DM // 128  # 8 contraction chunks for the projections
N_WARMUP = 10

PAIRSWAP = [i + 1 if i % 2 == 0 else i - 1 for i in range(32)]


def _install_ntff_hook_shim():
    """Register the axon NTFF profiling hook if antenv.axon_hooks is absent."""
    try:
        from antenv import axon_hooks  # noqa: F401
        return
    except ImportError:
        pass
    try:
        import antenv
        from trn_agent_boot.trn_boot import _ntff_profile_via_ctypes
        hook = _ntff_profile_via_ctypes('/opt/axon/libaxon_pjrt.so')
    except Exception:
        return
    mod = types.ModuleType('antenv.axon_hooks')
    mod._hook = hook
    mod.get_axon_ntff_profile_hook = lambda: mod._hook
    mod.set_axon_ntff_profile_hook = lambda h: setattr(mod, '_hook', h)
    sys.modules['antenv.axon_hooks'] = mod
    antenv.axon_hooks = mod


def build_program(s_len=S_FULL):
    """Build the single-core Bass program (identical across the 8 cores)."""
    nc = bacc.Bacc(None, target_bir_lowering=False, debug=False)

    nsc = s_len // SC  # number of 512-wide s-chunks

    # Host-prelaid: fat contiguous rows so each load is ONE DMA descriptor.
    xsH = nc.dram_tensor("xsH", [128, nsc * KCN * 512], BF16,
                         kind="ExternalInput").ap()
    wqkH = nc.dram_tensor("wqkH", [128, KCN * 1024], BF16,
                          kind="ExternalInput").ap()
    wvH = nc.dram_tensor("wvH", [128, KCN * 512], BF16,
                         kind="ExternalInput").ap()
    woH = nc.dram_tensor("woH", [128, 4 * 1024], BF16,
                         kind="ExternalInput").ap()
    cosA = nc.dram_tensor("cosA", [128, s_len], F32, kind="ExternalInput").ap()
    sinA = nc.dram_tensor("sinA", [128, s_len], F32, kind="ExternalInput").ap()
    triH = nc.dram_tensor("triH", [128, 128], BF16, kind="ExternalInput").ap()
    pat8 = nc.dram_tensor("pat8", [8, 512], F32R, kind="ExternalInput").ap()
    y = nc.dram_tensor("y", [s_len, DM], F32, kind="ExternalOutput").ap()

    with tile.TileContext(nc) as tc:
        with ExitStack() as ctx, nc.allow_low_precision(reason="bf16 kernel"):
            pers = ctx.enter_context(tc.tile_pool(name="pers", bufs=1))
            ps_pool = ctx.enter_context(
                tc.tile_pool(name="ps", bufs=1, space="PSUM"))
            xs_pool = ctx.enter_context(tc.tile_pool(name="xs", bufs=2))
            sh_pool = ctx.enter_context(tc.tile_pool(name="sh", bufs=3))
            p_pool = ctx.enter_context(tc.tile_pool(name="pp", bufs=5))
            oc_pool = ctx.enter_context(tc.tile_pool(name="oc", bufs=8))
            ocu_pool = ctx.enter_context(tc.tile_pool(name="ocu", bufs=2))
            rc_pool = ctx.enter_context(tc.tile_pool(name="rc", bufs=2))
            y_pool = ctx.enter_context(tc.tile_pool(name="yst", bufs=3))

            def psA(name):
                return ps_pool.tile([128, 1024], F32, tag="psA", bufs=3,
                                    name=name)

            qkT = [pers.tile([128, s_len], BF16, tag=f"qkT{t}", name=f"qkT{t}")
                   for t in range(8)]
            v_aug = [pers.tile([128, 8 * 65], BF16, tag=f"va{t}", name=f"va{t}")
                     for t in range(4 * nsc)]
            wqk = pers.tile([128, KCN * 1024], BF16, tag="wqk", name="wqk")
            wv = pers.tile([128, KCN * 512], BF16, tag="wv", name="wv")
            wo = pers.tile([128, 4 * 1024], BF16, tag="wo", name="wo")
            cosT = pers.tile([128, s_len], F32, tag="cos", name="cosT")
            sinT = pers.tile([128, s_len], F32, tag="sin", name="sinT")
            triT = pers.tile([128, 128], BF16, tag="tri", name="triT")
            pat8T = pers.tile([8, 512], F32R, tag="pat8", name="pat8T")
            wuS = pers.tile([128, 512], BF16, tag="wuS", name="wuS")
            junk = pers.tile([1, 16], BF16, tag="junk", name="junk")

            # ---- warmup (prime HAM + cover initial DMA latency) ----
            nc.vector.memset(wuS[:], 0.0)
            for i in range(N_WARMUP):
                wu = psA("wups")
                nc.tensor.matmul(wu[:, 0:512], wuS[:, 0:128], wuS[:],
                                 start=True, stop=True)
            # preload the ACT exp table during proj(0)
            nc.scalar.activation(junk[:], wuS[0:1, 0:16], AF.Exp, scale=SCALE)

            # ---- initial DMAs, priority-ordered per queue ----
            xs = {}

            def load_xs(sc):
                t = xs_pool.tile([128, KCN * 512], BF16, tag="xs", name="xs")
                nc.sync.dma_start(
                    t[:], xsH[:, 4096 * sc:4096 * (sc + 1)])
                xs[sc] = t

            load_xs(0)
            load_xs(1)
            nc.scalar.dma_start(wqk[:], wqkH[:])
            nc.scalar.dma_start(cosT[:], cosA[:])
            nc.scalar.dma_start(sinT[:], sinA[:])
            nc.gpsimd.dma_start(triT[:], triH[:])
            nc.gpsimd.dma_start(pat8T[:], pat8[:])
            nc.gpsimd.dma_start(wv[:], wvH[:])
            nc.gpsimd.dma_start(wo[:], woH[:])
            for vt in range(4 * nsc):
                v3 = v_aug[vt][:].rearrange("p (h c) -> p h c", c=65)
                nc.vector.memset(v3[:, :, 64:65], 1.0)

            # ---- projection work units ----
            def qk_unit(sc, gp):
                """One psum tile: projections for head-pair groups
                mg=2gp, 2gp+1 of s-chunk sc, plus their RoPE evacuation."""
                xt = xs[sc]
                ssl = slice(SC * sc, SC * (sc + 1))
                ps = psA("pjqk")
                for hc in range(2):
                    mg = 2 * gp + hc
                    half, mm = divmod(mg, 4)
                    c0 = 512 * hc
                    w0 = 512 * half + 128 * mm
                    for kc in range(KCN):
                        nc.tensor.matmul(
                            ps[:, c0:c0 + 512],
                            wqk[:, 1024 * kc + w0:1024 * kc + w0 + 128],
                            xt[:, 512 * kc:512 * (kc + 1)],
                            start=(kc == 0), stop=(kc == KCN - 1))
                for hc in range(2):
                    mg = 2 * gp + hc
                    c0 = 512 * hc
                    shuf = sh_pool.tile([128, SC], F32, tag="sh", name="shuf")
                    nc.vector.stream_shuffle(shuf[:], ps[:, c0:c0 + 512],
                                             PAIRSWAP)
                    nc.vector.tensor_mul(qkT[mg][:, ssl], ps[:, c0:c0 + 512],
                                         cosT[:, ssl])
                    nc.gpsimd.tensor_mul(shuf[:], shuf[:], sinT[:, ssl])
                    nc.vector.tensor_add(qkT[mg][:, ssl], qkT[mg][:, ssl],
                                         shuf[:])

            def v_unit(sc, gv):
                """One psum tile: v projection for sv=2gv, 2gv+1 of chunk sc."""
                xt = xs[sc]
                ps = psA("pjv")
                for hc in range(2):
                    sv = 2 * gv + hc
                    c0 = 512 * hc
                    for kc in range(KCN):
                        nc.tensor.matmul(
                            ps[:, c0:c0 + 512],
                            xt[:, 512 * kc + 128 * sv:
                               512 * kc + 128 * (sv + 1)],
                            wv[:, 512 * kc:512 * (kc + 1)],
                            start=(kc == 0), stop=(kc == KCN - 1))
                for hc in range(2):
                    sv = 2 * gv + hc
                    vt = 4 * sc + sv
                    v3 = v_aug[vt][:].rearrange("p (h c) -> p h c", c=65)
                    nc.scalar.copy(
                        v3[:, :, 0:64],
                        ps[:, 512 * hc:512 * (hc + 1)].rearrange(
                            "p (h c) -> p h c", c=64))

            def proj_units(sc):
                us = [lambda gp=gp: qk_unit(sc, gp) for gp in range(4)]
                us += [lambda gv=gv: v_unit(sc, gv) for gv in range(2)]
                return us

            # ---- attention ----
            def attention_qc(qc, ocU_all, fillers):
                nblk = 4 * qc + 4
                outT = {}
                sc_ps = {}

                def q0_of(kb):
                    j = kb - 4 * qc
                    return 128 * j if j >= 0 else 0

                def emit_scores(p, kb):
                    qT, kT = qkT[p], qkT[4 + p]
                    q0 = q0_of(kb)
                    ksl = slice(128 * kb, 128 * (kb + 1))
                    ps = psA("scps")
                    nc.tensor.matmul(
                        ps[:, q0:512], kT[0:64, ksl],
                        qT[0:64, SC * qc + q0:SC * (qc + 1)],
                        start=True, stop=True, tile_position=(0, 0))
                    nc.tensor.matmul(
                        ps[:, 512 + q0:1024], kT[64:128, ksl],
                        qT[64:128, SC * qc + q0:SC * (qc + 1)],
                        start=True, stop=True, tile_position=(64, 0))
                    sc_ps[p, kb] = ps

                pP = {}

                def emit_exp_mask(p, kb):
                    """exp + diagonal mask for step (p, kb); the PV matmuls
                    run one step later so ACT never gates the PE."""
                    q0 = q0_of(kb)
                    j = kb - 4 * qc
                    ps = sc_ps.pop((p, kb))
                    P = p_pool.tile([128, 1024], BF16, tag="P", name="Pt")
                    vps = ps[:].rearrange("p (two q) -> p two q", two=2)
                    vP = P[:].rearrange("p (two q) -> p two q", two=2)
                    nc.scalar.activation(vP[:, :, q0:512], vps[:, :, q0:512],
                                         AF.Exp, scale=SCALE)
                    if j >= 0:
                        # mask only the diagonal 128x128 block
                        nc.vector.tensor_mul(P[:, q0:q0 + 128],
                                             P[:, q0:q0 + 128], triT[:])
                        nc.vector.tensor_mul(P[:, 512 + q0:512 + q0 + 128],
                                             P[:, 512 + q0:512 + q0 + 128],
                                             triT[:])
                    pP[p, kb] = P

                def emit_pv(p, kb):
                    q0 = q0_of(kb)
                    P = pP.pop((p, kb))
                    if kb == 0:
                        outT[p, 0] = ps_pool.tile([65, SC], F32, tag="outT",
                                                  bufs=2, name="outA")
                        outT[p, 1] = ps_pool.tile([65, SC], F32, tag="outT",
                                                  bufs=2, name="outB")
                    va = v_aug[kb]
                    nc.tensor.matmul(
                        outT[p, 0][:, q0:512], va[:, 130 * p:130 * p + 65],
                        P[:, q0:512],
                        start=(kb == 0), stop=(kb == nblk - 1))
                    nc.tensor.matmul(
                        outT[p, 1][:, q0:512],
                        va[:, 130 * p + 65:130 * p + 130],
                        P[:, 512 + q0:1024],
                        start=(kb == 0), stop=(kb == nblk - 1))
                    if kb == nblk - 1:
                        for h in range(2):
                            i = 2 * p + h
                            nc.vector.tensor_copy(
                                ocU_all[:, 512 * i:512 * (i + 1)],
                                outT.pop((p, h))[:])

                stream = [(p, kb) for p in range(4) for kb in range(nblk)]
                iv = max(2, (len(stream) + len(fillers)) // (len(fillers) + 1)) \
                    if fillers else 10 ** 9
                emitted = 0
                for idx, (p, kb) in enumerate(stream):
                    while emitted <= idx + 2 and emitted < len(stream):
                        emit_scores(*stream[emitted])
                        emitted += 1
                    emit_exp_mask(p, kb)
                    if idx >= 1:
                        emit_pv(*stream[idx - 1])
                    if fillers and idx % iv == iv - 1:
                        fillers.popleft()()
                emit_pv(*stream[-1])

            def normalize(qc, ocU_all):
                """DVE reciprocal of the 8 staged denominator rows, then a
                K=8 selector matmul broadcast + the normalize muls."""
                dn8 = rc_pool.tile([8, SC], BF16, tag="dn8", name="dn8")
                for i in range(8):
                    nc.gpsimd.dma_start(dn8[i:i + 1, :],
                                        ocU_all[64:65, 512 * i:512 * (i + 1)])
                dnf = rc_pool.tile([8, SC], F32, tag="dnf", name="dnf")
                nc.vector.tensor_copy(dnf[:], dn8[:])
                rcp = rc_pool.tile([8, SC], F32, tag="rcp", name="rcp")
                nc.vector.reciprocal_approx_fast(rcp[:], dnf[:])
                rcpR = rc_pool.tile([8, SC], F32R, tag="rcpR", name="rcpR")
                nc.vector.tensor_copy(rcpR[:], rcp[:])
                oc_t = [oc_pool.tile([128, SC], BF16, tag="oc", name="oc")
                        for _ in range(4)]
                for p in range(4):
                    bcq = psA("bcq")
                    nc.tensor.matmul(bcq[:, 0:SC],
                                     pat8T[:, 128 * p:128 * (p + 1)],
                                     rcpR[:], start=True, stop=True)
                    nc.vector.tensor_mul(
                        oc_t[p][0:64, :],
                        ocU_all[0:64, 1024 * p:1024 * p + 512],
                        bcq[0:64, 0:SC])
                    nc.vector.tensor_mul(
                        oc_t[p][64:128, :],
                        ocU_all[0:64, 1024 * p + 512:1024 * (p + 1)],
                        bcq[64:128, 0:SC])
                return oc_t

            def outproj_chunk(qc, oc_t, sv):
                svsl = slice(128 * sv, 128 * (sv + 1))
                ps = psA("psy")
                for k in range(4):
                    nc.tensor.matmul(ps[:, 0:512], oc_t[k][:, svsl],
                                     wo[:, 1024 * k:1024 * k + 512],
                                     start=(k == 0), stop=(k == 3))
                    nc.tensor.matmul(ps[:, 512:1024], oc_t[k][:, svsl],
                                     wo[:, 1024 * k + 512:1024 * (k + 1)],
                                     start=(k == 0), stop=(k == 3))
                yt = y_pool.tile([128, 1024], F32, tag="yst", name="yt")
                nc.vector.tensor_copy(yt[:], ps[:])
                nc.sync.dma_start(
                    y[SC * qc + 128 * sv:SC * qc + 128 * (sv + 1), :],
                    yt[:])

            # ---- fused schedule ----
            for u in proj_units(0):
                u()
            fillers = deque()
            pending = None
            for qc in range(nsc):
                if qc + 2 < nsc:
                    fillers.append(lambda sc=qc + 2: load_xs(sc))
                if pending is not None:
                    pqc, pocU = pending
                    oc_t = normalize(pqc, pocU)
                    fillers.extend(
                        (lambda sv=sv, q=pqc, o=oc_t:
                         outproj_chunk(q, o, sv)) for sv in range(4))
                    pending = None
                if qc + 1 < nsc:
                    fillers.extend(proj_units(qc + 1))
                ocU_all = ocu_pool.tile([65, 8 * SC], BF16, tag="ocu",
                                        name="ocu")
                attention_qc(qc, ocU_all, fillers)
                pending = (qc, ocU_all)
            while fillers:
                fillers.popleft()()
            pqc, pocU = pending
            oc_t = normalize(pqc, pocU)
            for sv in range(4):
                outproj_chunk(pqc, oc_t, sv)

    nc.compile()
    return nc


# ---------------------------------------------------------------------------
# Host-side input preparation
# ---------------------------------------------------------------------------

BF = ml_dtypes.bfloat16


def _rope_tables(s_len):
    perm = np.empty(64, dtype=np.int64)
    perm[0::2] = np.arange(32)
    perm[1::2] = np.arange(32) + 32
    inv_freq = 1.0 / (ROPE_BASE ** (np.arange(0, D, 2, dtype=np.float32) / D))
    t = np.arange(s_len, dtype=np.float32)
    freqs = np.einsum('i,j->ij', t, inv_freq)           # [S, 32]
    emb = np.concatenate([freqs, freqs], axis=-1)       # [S, 64]
    cos = np.cos(emb).T.astype(np.float32)              # [64, S]
    sin = np.sin(emb).T.astype(np.float32)
    cos64 = cos[perm]
    sin64 = sin[perm]
    sign = np.where(perm < 32, -1.0, 1.0).astype(np.float32)[:, None]
    sin64 = sin64 * sign
    cosA = np.ascontiguousarray(np.tile(cos64, (2, 1)))
    sinA = np.ascontiguousarray(np.tile(sin64, (2, 1)))
    return perm, cosA, sinA


def _chunk128(a):
    """[N*128, M] -> [128, N*M] with N-major column blocks."""
    n = a.shape[0] // 128
    return np.ascontiguousarray(
        a.reshape(n, 128, a.shape[1]).transpose(1, 0, 2).reshape(
            128, n * a.shape[1]))


def make_in_maps(x, W_qkv, W_out, s_len=S_FULL):
    B = x.shape[0]
    nsc = s_len // SC
    perm, cosA, sinA = _rope_tables(s_len)
    tri = np.triu(np.ones((128, 128), dtype=np.float32)).astype(BF)
    pat = np.zeros((8, 512), dtype=np.float32)
    for p in range(4):
        pat[2 * p, 128 * p:128 * p + 64] = 1.0
        pat[2 * p + 1, 128 * p + 64:128 * (p + 1)] = 1.0
    in_maps = []
    for c in range(2 * B):
        b, hg = c // 2, c % 2
        # xsH[p, sc*4096 + kc*512 + q] = x[b, 512*sc + q, 128*kc + p]
        xb = x[b, :s_len].reshape(nsc, 512, KCN, 128)     # [sc, q, kc, p]
        xsH = np.ascontiguousarray(
            xb.transpose(3, 0, 2, 1).reshape(128, nsc * KCN * 512).astype(BF))
        cols = []
        for h in range(HPC):
            cols.append(W_qkv[64 * (HPC * hg + h) + perm])          # q head
        for h in range(HPC):
            cols.append(W_qkv[1024 + 64 * (HPC * hg + h) + perm])   # k head
        wqkT = np.concatenate(cols, axis=0).T.astype(BF)  # [1024, 1024]
        wvT = W_qkv[2048 + 512 * hg:2048 + 512 * (hg + 1)].T.astype(BF)
        woT = W_out[:, 512 * hg:512 * (hg + 1)].T.astype(BF)
        in_maps.append({
            "xsH": xsH,
            "wqkH": _chunk128(wqkT),
            "wvH": _chunk128(wvT),
            "woH": _chunk128(woT),
            "cosA": cosA, "sinA": sinA, "triH": tri,
            "pat8": pat,
        })
    return in_maps


_NC_CACHE = {}


def _get_program(s_len=S_FULL):
    if s_len not in _NC_CACHE:
        _NC_CACHE[s_len] = build_program(s_len)
    return _NC_CACHE[s_len]


def kernel(x, W_qkv, W_out):
    """Full-input, full-output causal self-attention on 8 NeuronCores."""
    _install_ntff_hook_shim()
    x = np.asarray(x, dtype=np.float32)
    W_qkv = np.asarray(W_qkv, dtype=np.float32)
    W_out = np.asarray(W_out, dtype=np.float32)
    B, S, dm = x.shape

    nc = _get_program(S)
    in_maps = make_in_maps(x, W_qkv, W_out, S)
    res = bass_utils.run_bass_kernel_spmd(nc, in_maps, list(range(2 * B)))
    out = np.empty((B, S, dm), dtype=np.float32)
    for b in range(B):
        out[b] = res.results[2 * b]["y"] + res.results[2 * b + 1]["y"]
    return out


# revision 13
# speedup vs baseline: 1.3131x; 1.0581x over previous
"""Trainium2 Bass kernel: causal self-attention with RoPE.

Problem: x[4, 2048, 1024], W_qkv[3072, 1024], W_out[1024, 1024], 16 heads.
Sharding: 8 cores = (batch b, head-group hg of 8 heads); core c -> b=c//2,
hg=c%2. Each core computes a full [S, d_model] partial of the output (its
8 heads' contribution through out_proj); the host sums the two head-group
partials per batch.

On-chip layout is fully "transposed": q^T/k^T are produced as [d, s] tiles
(two heads per 128-partition tile), scores are computed as S^T = [k, q] so
the softmax needs no on-chip transposes, and PV/out_proj consume the
transposed forms directly, producing y in natural [s, e] layout.

V2: single fused pipeline. The QKV projection for s-chunk sc+1, the
normalize/out_proj of chunk qc-1, and the x-prefetch DMAs are dripped as
"filler" work units between the attention steps of chunk qc, so the PE
never sits idle while ACT runs the softmax exps (which are the true
bottleneck: ACT is 1 elem/cycle/lane @ 1.2 GHz, ~160us of exp total).
Everything on-chip is bf16 except psum/rope/y staging; weights live
resident in SBUF and all big inputs are host-prelaid so each load is one
fat-row DMA descriptor. Warmup matmuls at t=0 prime the HAM clock gate.
"""

import sys
import types
from collections import deque
from contextlib import ExitStack

import numpy as np
import ml_dtypes

import concourse.bass as bass
import concourse.mybir as mybir
import concourse.tile as tile
from concourse import bacc, bass_utils

F32 = mybir.dt.float32
F32R = mybir.dt.float32r
BF16 = mybir.dt.bfloat16
AF = mybir.ActivationFunctionType

N_HEADS = 16
ROPE_BASE = 10000.0
B_FULL, S_FULL, DM = 4, 2048, 1024
HPC = 8          # heads per core
D = 64           # head dim
SCALE = 1.0 / 8.0  # D ** -0.5
SC = 512         # s-chunk width
KCN = DM // 128  # 8 contraction chunks for the projections
N_WARMUP = 10

PAIRSWAP = [i + 1 if i % 2 == 0 else i - 1 for i in range(32)]


def _install_ntff_hook_shim():
    """Register the axon NTFF profiling hook if antenv.axon_hooks is absent."""
    try:
        from antenv import axon_hooks  # noqa: F401
        return
    except ImportError:
        pass
    try:
        import antenv
        from trn_agent_boot.trn_boot import _ntff_profile_via_ctypes
        hook = _ntff_profile_via_ctypes('/opt/axon/libaxon_pjrt.so')
    except Exception:
        return
    mod = types.ModuleType('antenv.axon_hooks')
    mod._hook = hook
    mod.get_axon_ntff_profile_hook = lambda: mod._hook
    mod.set_axon_ntff_profile_hook = lambda h: setattr(mod, '_hook', h)
    sys.modules['antenv.axon_hooks'] = mod
    antenv.axon_hooks = mod


def build_program(s_len=S_FULL):
    """Build the single-core Bass program (identical across the 8 cores)."""
    nc = bacc.Bacc(None, target_bir_lowering=False, debug=False)

    nsc = s_len // SC  # number of 512-wide s-chunks

    # Host-prelaid: fat contiguous rows so each load is ONE DMA descriptor.
    xsH = nc.dram_tensor("xsH", [128, nsc * KCN * 512], BF16,
                         kind="ExternalInput").ap()
    wqkH = nc.dram_tensor("wqkH", [128, KCN * 1024], BF16,
                          kind="ExternalInput").ap()
    wvH = nc.dram_tensor("wvH", [128, KCN * 512], BF16,
                         kind="ExternalInput").ap()
    woH = nc.dram_tensor("woH", [128, 4 * 1024], BF16,
                         kind="ExternalInput").ap()
    cosA = nc.dram_tensor("cosA", [128, s_len], F32, kind="ExternalInput").ap()
    sinA = nc.dram_tensor("sinA", [128, s_len], F32, kind="ExternalInput").ap()
    triH = nc.dram_tensor("triH", [128, 128], BF16, kind="ExternalInput").ap()
    pat8 = nc.dram_tensor("pat8", [8, 512], F32R, kind="ExternalInput").ap()
    y = nc.dram_tensor("y", [s_len, DM], F32, kind="ExternalOutput").ap()

    with tile.TileContext(nc) as tc:
        with ExitStack() as ctx, nc.allow_low_precision(reason="bf16 kernel"):
            pers = ctx.enter_context(tc.tile_pool(name="pers", bufs=1))
            ps_pool = ctx.enter_context(
                tc.tile_pool(name="ps", bufs=1, space="PSUM"))
            xs_pool = ctx.enter_context(tc.tile_pool(name="xs", bufs=2))
            sh_pool = ctx.enter_context(tc.tile_pool(name="sh", bufs=3))
            p_pool = ctx.enter_context(tc.tile_pool(name="pp", bufs=5))
            oc_pool = ctx.enter_context(tc.tile_pool(name="oc", bufs=8))
            ocu_pool = ctx.enter_context(tc.tile_pool(name="ocu", bufs=2))
            rc_pool = ctx.enter_context(tc.tile_pool(name="rc", bufs=2))
            y_pool = ctx.enter_context(tc.tile_pool(name="yst", bufs=3))

            def psA(name):
                return ps_pool.tile([128, 1024], F32, tag="psA", bufs=3,
                                    name=name)

            qkT = [pers.tile([128, s_len], BF16, tag=f"qkT{t}", name=f"qkT{t}")
                   for t in range(8)]
            v_aug = [pers.tile([128, 8 * 65], BF16, tag=f"va{t}", name=f"va{t}")
                     for t in range(4 * nsc)]
            wqk = pers.tile([128, KCN * 1024], BF16, tag="wqk", name="wqk")
            wv = pers.tile([128, KCN * 512], BF16, tag="wv", name="wv")
            wo = pers.tile([128, 4 * 1024], BF16, tag="wo", name="wo")
            cosT = pers.tile([128, s_len], F32, tag="cos", name="cosT")
            sinT = pers.tile([128, s_len], F32, tag="sin", name="sinT")
            triT = pers.tile([128, 128], BF16, tag="tri", name="triT")
            pat8T = pers.tile([8, 512], F32R, tag="pat8", name="pat8T")
            wuS = pers.tile([128, 512], BF16, tag="wuS", name="wuS")
            junk = pers.tile([1, 16], BF16, tag="junk", name="junk")

            # ---- warmup (prime HAM + cover initial DMA latency) ----
            nc.vector.memset(wuS[:], 0.0)
            for i in range(N_WARMUP):
                wu = psA("wups")
                nc.tensor.matmul(wu[:, 0:512], wuS[:, 0:128], wuS[:],
                                 start=True, stop=True)
            # preload the ACT exp table during proj(0)
            nc.scalar.activation(junk[:], wuS[0:1, 0:16], AF.Exp, scale=SCALE)

            # ---- initial DMAs, priority-ordered per queue ----
            xs = {}

            def load_xs(sc):
                t = xs_pool.tile([128, KCN * 512], BF16, tag="xs", name="xs")
                nc.sync.dma_start(
                    t[:], xsH[:, 4096 * sc:4096 * (sc + 1)])
                xs[sc] = t

            load_xs(0)
            load_xs(1)
            nc.scalar.dma_start(wqk[:], wqkH[:])
            nc.scalar.dma_start(cosT[:], cosA[:])
            nc.scalar.dma_start(sinT[:], sinA[:])
            nc.gpsimd.dma_start(triT[:], triH[:])
            nc.gpsimd.dma_start(pat8T[:], pat8[:])
            nc.gpsimd.dma_start(wv[:], wvH[:])
            nc.gpsimd.dma_start(wo[:], woH[:])
            for vt in range(4 * nsc):
                v3 = v_aug[vt][:].rearrange("p (h c) -> p h c", c=65)
                nc.vector.memset(v3[:, :, 64:65], 1.0)

            # ---- projection work units (one 8-matmul accumulation group +
            # its evacuation per filler, so a dripped unit never starves the
            # ACT exp backlog) ----
            def qk_unit(sc, mg):
                xt = xs[sc]
                ssl = slice(SC * sc, SC * (sc + 1))
                ps = psA("pjqk")
                half, mm = divmod(mg, 4)
                w0 = 512 * half + 128 * mm
                for kc in range(KCN):
                    nc.tensor.matmul(
                        ps[:, 0:512],
                        wqk[:, 1024 * kc + w0:1024 * kc + w0 + 128],
                        xt[:, 512 * kc:512 * (kc + 1)],
                        start=(kc == 0), stop=(kc == KCN - 1))
                shuf = sh_pool.tile([128, SC], F32, tag="sh", name="shuf")
                nc.vector.stream_shuffle(shuf[:], ps[:, 0:512], PAIRSWAP)
                nc.vector.tensor_mul(qkT[mg][:, ssl], ps[:, 0:512],
                                     cosT[:, ssl])
                nc.gpsimd.tensor_mul(shuf[:], shuf[:], sinT[:, ssl])
                nc.vector.tensor_add(qkT[mg][:, ssl], qkT[mg][:, ssl],
                                     shuf[:])

            def v_unit(sc, sv):
                xt = xs[sc]
                ps = psA("pjv")
                for kc in range(KCN):
                    nc.tensor.matmul(
                        ps[:, 0:512],
                        xt[:, 512 * kc + 128 * sv:512 * kc + 128 * (sv + 1)],
                        wv[:, 512 * kc:512 * (kc + 1)],
                        start=(kc == 0), stop=(kc == KCN - 1))
                vt = 4 * sc + sv
                v3 = v_aug[vt][:].rearrange("p (h c) -> p h c", c=65)
                nc.scalar.copy(
                    v3[:, :, 0:64],
                    ps[:, 0:512].rearrange("p (h c) -> p h c", c=64))

            def proj_units(sc):
                us = [lambda mg=mg: qk_unit(sc, mg) for mg in range(8)]
                us += [lambda sv=sv: v_unit(sc, sv) for sv in range(4)]
                return us

            # ---- attention ----
            def attention_qc(qc, ocU_all, fillers):
                nblk = 4 * qc + 4
                outT = {}
                sc_ps = {}

                def q0_of(kb):
                    j = kb - 4 * qc
                    return 128 * j if j >= 0 else 0

                def emit_scores(p, kb):
                    qT, kT = qkT[p], qkT[4 + p]
                    q0 = q0_of(kb)
                    ksl = slice(128 * kb, 128 * (kb + 1))
                    ps = psA("scps")
                    nc.tensor.matmul(
                        ps[:, q0:512], kT[0:64, ksl],
                        qT[0:64, SC * qc + q0:SC * (qc + 1)],
                        start=True, stop=True, tile_position=(0, 0))
                    nc.tensor.matmul(
                        ps[:, 512 + q0:1024], kT[64:128, ksl],
                        qT[64:128, SC * qc + q0:SC * (qc + 1)],
                        start=True, stop=True, tile_position=(64, 0))
                    sc_ps[p, kb] = ps

                pP = {}

                def emit_exp_mask(p, kb):
                    """exp + diagonal mask for step (p, kb); the PV matmuls
                    run one step later so ACT never gates the PE."""
                    q0 = q0_of(kb)
                    j = kb - 4 * qc
                    ps = sc_ps.pop((p, kb))
                    P = p_pool.tile([128, 1024], BF16, tag="P", name="Pt")
                    vps = ps[:].rearrange("p (two q) -> p two q", two=2)
                    vP = P[:].rearrange("p (two q) -> p two q", two=2)
                    nc.scalar.activation(vP[:, :, q0:512], vps[:, :, q0:512],
                                         AF.Exp, scale=SCALE)
                    if j >= 0:
                        # mask only the diagonal 128x128 block
                        nc.vector.tensor_mul(P[:, q0:q0 + 128],
                                             P[:, q0:q0 + 128], triT[:])
                        nc.vector.tensor_mul(P[:, 512 + q0:512 + q0 + 128],
                                             P[:, 512 + q0:512 + q0 + 128],
                                             triT[:])
                    pP[p, kb] = P

                def emit_pv(p, kb):
                    q0 = q0_of(kb)
                    P = pP.pop((p, kb))
                    if kb == 0:
                        outT[p, 0] = ps_pool.tile([65, SC], F32, tag="outT",
                                                  bufs=2, name="outA")
                        outT[p, 1] = ps_pool.tile([65, SC], F32, tag="outT",
                                                  bufs=2, name="outB")
                    va = v_aug[kb]
                    nc.tensor.matmul(
                        outT[p, 0][:, q0:512], va[:, 130 * p:130 * p + 65],
                        P[:, q0:512],
                        start=(kb == 0), stop=(kb == nblk - 1))
                    nc.tensor.matmul(
                        outT[p, 1][:, q0:512],
                        va[:, 130 * p + 65:130 * p + 130],
                        P[:, 512 + q0:1024],
                        start=(kb == 0), stop=(kb == nblk - 1))
                    if kb == nblk - 1:
                        for h in range(2):
                            i = 2 * p + h
                            nc.vector.tensor_copy(
                                ocU_all[:, 512 * i:512 * (i + 1)],
                                outT.pop((p, h))[:])

                stream = [(p, kb) for p in range(4) for kb in range(nblk)]
                # fractional drip pacing; all fillers MUST drain within this
                # chunk (attention of chunk qc+1 reads what they produce)
                rate = len(fillers) / max(1, len(stream) - 2)
                credit = 0.0
                emitted = 0
                for idx, (p, kb) in enumerate(stream):
                    while emitted <= idx + 2 and emitted < len(stream):
                        emit_scores(*stream[emitted])
                        emitted += 1
                    emit_exp_mask(p, kb)
                    if idx >= 1:
                        emit_pv(*stream[idx - 1])
                    credit += rate
                    while fillers and credit >= 1.0:
                        fillers.popleft()()
                        credit -= 1.0
                emit_pv(*stream[-1])
                while fillers:
                    fillers.popleft()()

            def normalize(qc, ocU_all):
                """DVE reciprocal of the 8 staged denominator rows, then a
                K=8 selector matmul broadcast + the normalize muls."""
                dn8 = rc_pool.tile([8, SC], BF16, tag="dn8", name="dn8")
                for i in range(8):
                    nc.gpsimd.dma_start(dn8[i:i + 1, :],
                                        ocU_all[64:65, 512 * i:512 * (i + 1)])
                dnf = rc_pool.tile([8, SC], F32, tag="dnf", name="dnf")
                nc.vector.tensor_copy(dnf[:], dn8[:])
                rcp = rc_pool.tile([8, SC], F32, tag="rcp", name="rcp")
                nc.vector.reciprocal_approx_fast(rcp[:], dnf[:])
                rcpR = rc_pool.tile([8, SC], F32R, tag="rcpR", name="rcpR")
                nc.vector.tensor_copy(rcpR[:], rcp[:])
                oc_t = [oc_pool.tile([128, SC], BF16, tag="oc", name="oc")
                        for _ in range(4)]
                for p in range(4):
                    bcq = psA("bcq")
                    nc.tensor.matmul(bcq[:, 0:SC],
                                     pat8T[:, 128 * p:128 * (p + 1)],
                                     rcpR[:], start=True, stop=True)
                    nc.vector.tensor_mul(
                        oc_t[p][0:64, :],
                        ocU_all[0:64, 1024 * p:1024 * p + 512],
                        bcq[0:64, 0:SC])
                    nc.vector.tensor_mul(
                        oc_t[p][64:128, :],
                        ocU_all[0:64, 1024 * p + 512:1024 * (p + 1)],
                        bcq[64:128, 0:SC])
                return oc_t

            def outproj_chunk(qc, oc_t, sv):
                svsl = slice(128 * sv, 128 * (sv + 1))
                ps = psA("psy")
                for k in range(4):
                    nc.tensor.matmul(ps[:, 0:512], oc_t[k][:, svsl],
                                     wo[:, 1024 * k:1024 * k + 512],
                                     start=(k == 0), stop=(k == 3))
                    nc.tensor.matmul(ps[:, 512:1024], oc_t[k][:, svsl],
                                     wo[:, 1024 * k + 512:1024 * (k + 1)],
                                     start=(k == 0), stop=(k == 3))
                yt = y_pool.tile([128, 1024], F32, tag="yst", name="yt")
                nc.vector.tensor_copy(yt[:], ps[:])
                nc.sync.dma_start(
                    y[SC * qc + 128 * sv:SC * qc + 128 * (sv + 1), :],
                    yt[:])

            # ---- fused schedule ----
            for u in proj_units(0):
                u()
            fillers = deque()
            pending = None
            for qc in range(nsc):
                if qc + 2 < nsc:
                    fillers.append(lambda sc=qc + 2: load_xs(sc))
                if pending is not None:
                    pqc, pocU = pending
                    holder = {}

                    def norm_unit(q=pqc, u=pocU, h=holder):
                        h['oc'] = normalize(q, u)

                    fillers.append(norm_unit)
                    fillers.extend(
                        (lambda sv=sv, q=pqc, h=holder:
                         outproj_chunk(q, h['oc'], sv)) for sv in range(4))
                    pending = None
                if qc + 1 < nsc:
                    fillers.extend(proj_units(qc + 1))
                ocU_all = ocu_pool.tile([65, 8 * SC], BF16, tag="ocu",
                                        name="ocu")
                attention_qc(qc, ocU_all, fillers)
                pending = (qc, ocU_all)
            pqc, pocU = pending
            oc_t = normalize(pqc, pocU)
            for sv in range(4):
                outproj_chunk(pqc, oc_t, sv)

    nc.compile()
    return nc


# ---------------------------------------------------------------------------
# Host-side input preparation
# ---------------------------------------------------------------------------

BF = ml_dtypes.bfloat16


def _rope_tables(s_len):
    perm = np.empty(64, dtype=np.int64)
    perm[0::2] = np.arange(32)
    perm[1::2] = np.arange(32) + 32
    inv_freq = 1.0 / (ROPE_BASE ** (np.arange(0, D, 2, dtype=np.float32) / D))
    t = np.arange(s_len, dtype=np.float32)
    freqs = np.einsum('i,j->ij', t, inv_freq)           # [S, 32]
    emb = np.concatenate([freqs, freqs], axis=-1)       # [S, 64]
    cos = np.cos(emb).T.astype(np.float32)              # [64, S]
    sin = np.sin(emb).T.astype(np.float32)
    cos64 = cos[perm]
    sin64 = sin[perm]
    sign = np.where(perm < 32, -1.0, 1.0).astype(np.float32)[:, None]
    sin64 = sin64 * sign
    cosA = np.ascontiguousarray(np.tile(cos64, (2, 1)))
    sinA = np.ascontiguousarray(np.tile(sin64, (2, 1)))
    return perm, cosA, sinA


def _chunk128(a):
    """[N*128, M] -> [128, N*M] with N-major column blocks."""
    n = a.shape[0] // 128
    return np.ascontiguousarray(
        a.reshape(n, 128, a.shape[1]).transpose(1, 0, 2).reshape(
            128, n * a.shape[1]))


def make_in_maps(x, W_qkv, W_out, s_len=S_FULL):
    B = x.shape[0]
    nsc = s_len // SC
    perm, cosA, sinA = _rope_tables(s_len)
    tri = np.triu(np.ones((128, 128), dtype=np.float32)).astype(BF)
    pat = np.zeros((8, 512), dtype=np.float32)
    for p in range(4):
        pat[2 * p, 128 * p:128 * p + 64] = 1.0
        pat[2 * p + 1, 128 * p + 64:128 * (p + 1)] = 1.0
    in_maps = []
    for c in range(2 * B):
        b, hg = c // 2, c % 2
        # xsH[p, sc*4096 + kc*512 + q] = x[b, 512*sc + q, 128*kc + p]
        xb = x[b, :s_len].reshape(nsc, 512, KCN, 128)     # [sc, q, kc, p]
        xsH = np.ascontiguousarray(
            xb.transpose(3, 0, 2, 1).reshape(128, nsc * KCN * 512).astype(BF))
        cols = []
        for h in range(HPC):
            cols.append(W_qkv[64 * (HPC * hg + h) + perm])          # q head
        for h in range(HPC):
            cols.append(W_qkv[1024 + 64 * (HPC * hg + h) + perm])   # k head
        wqkT = np.concatenate(cols, axis=0).T.astype(BF)  # [1024, 1024]
        wvT = W_qkv[2048 + 512 * hg:2048 + 512 * (hg + 1)].T.astype(BF)
        woT = W_out[:, 512 * hg:512 * (hg + 1)].T.astype(BF)
        in_maps.append({
            "xsH": xsH,
            "wqkH": _chunk128(wqkT),
            "wvH": _chunk128(wvT),
            "woH": _chunk128(woT),
            "cosA": cosA, "sinA": sinA, "triH": tri,
            "pat8": pat,
        })
    return in_maps


_NC_CACHE = {}


def _get_program(s_len=S_FULL):
    if s_len not in _NC_CACHE:
        _NC_CACHE[s_len] = build_program(s_len)
    return _NC_CACHE[s_len]


def kernel(x, W_qkv, W_out):
    """Full-input, full-output causal self-attention on 8 NeuronCores."""
    _install_ntff_hook_shim()
    x = np.asarray(x, dtype=np.float32)
    W_qkv = np.asarray(W_qkv, dtype=np.float32)
    W_out = np.asarray(W_out, dtype=np.float32)
    B, S, dm = x.shape

    nc = _get_program(S)
    in_maps = make_in_maps(x, W_qkv, W_out, S)
    res = bass_utils.run_bass_kernel_spmd(nc, in_maps, list(range(2 * B)))
    out = np.empty((B, S, dm), dtype=np.float32)
    for b in range(B):
        out[b] = res.results[2 * b]["y"] + res.results[2 * b + 1]["y"]
    return out
